# revision 11
# baseline (speedup 1.0000x reference)
"""Quaternion batch-norm (nn_BatchNormalizationQ) Trainium2 kernel.

Strategy (8 NeuronCores, batch-parallel):
  - Host shards x [4,32,56,56,256] on batch -> per core [4, 12544, 256]
    (component, spatial, channel); gamma/beta pre-transposed to [256,*].
  - Phase 1 (stats): cast-DMA tiles to bf16, PE computes per-channel
    Gram sums  sum_s x_p x_q  (10 pairs) and sums  sum_s x_p  (via
    ones-vector matmuls), accumulated in PSUM.  Diagonals extracted with
    identity-mask multiply + row-reduce.  Partial sums [128,28] are
    AllReduced across the 8 cores.
  - Whitening: per-channel 4x4 inverse-Cholesky + gamma fusion computed
    on-chip on [128,2] tiles (channel on partitions), giving M[4][4] and
    b' = beta - M mu as per-partition scalar vectors.
  - Phase 2 (apply): fp32 tiles, PE-transposed to channel-major strips in
    PSUM, drained to SBUF by ACT; out_q = ((x0*M_q0+b'_q) chained with
    scalar_tensor_tensor FMAs on DVE.  Output written channel-major
    [4, 256, 12544] per core; host transposes back.
"""
import numpy as np

from concourse import bass, bacc, tile, mybir
from concourse.bass_utils import run_bass_kernel_spmd

F32 = mybir.dt.float32
BF16 = mybir.dt.bfloat16
AOP = mybir.AluOpType
AF = mybir.ActivationFunctionType

P = 128
C = 256          # channels
NCOMP = 4        # quaternion components
EPS = 1e-4
CHUNK = 1024     # spatial rows per chunk (must be multiple of 128)

NAMES = "rijk"
TRI = [(p1, p2) for p1 in range(4) for p2 in range(p1, 4)]
TRI_IDX = {}
for _i, (_p, _q) in enumerate(TRI):
    TRI_IDX[(_p, _q)] = _i
    TRI_IDX[(_q, _p)] = _i


def _chunks(S):
    out = []
    s0 = 0
    while s0 < S:
        rows = min(CHUNK, S - s0)
        out.append((s0, rows))
        s0 += rows
    return out


def build_bass(S, n_cores, debug_out=False):
    """Build the SPMD program for per-core spatial size S over n_cores."""
    NTOT = float(S * n_cores)
    nc = bacc.Bacc("TRN2", target_bir_lowering=False, debug=False,
                   num_devices=n_cores)

    x_dram = nc.dram_tensor("x", [NCOMP, S, C], F32, kind="ExternalInput")
    gam_dram = nc.dram_tensor("gammaT", [C, 10], F32, kind="ExternalInput")
    beta_dram = nc.dram_tensor("betaT", [C, NCOMP], F32, kind="ExternalInput")
    id_dram = nc.dram_tensor("ident", [P, P], F32, kind="ExternalInput")
    out_dram = nc.dram_tensor("out_t", [NCOMP, C, S], F32, kind="ExternalOutput")
    if debug_out:
        dbg_stats = nc.dram_tensor("dbg_stats", [P, 28], F32, kind="ExternalOutput")
        dbg_m = nc.dram_tensor("dbg_m", [P, 2, 16], F32, kind="ExternalOutput")
        dbg_bp = nc.dram_tensor("dbg_bp", [P, 2, 4], F32, kind="ExternalOutput")

    chunks = _chunks(S)
    last_ci = len(chunks) - 1

    with tile.TileContext(nc) as tc:
        import contextlib
        stack = contextlib.ExitStack()
        with stack:
            const_pool = stack.enter_context(tc.tile_pool(name="consts", bufs=1))
            wh_pool = stack.enter_context(tc.tile_pool(name="whiten", bufs=1))
            dram_pool = stack.enter_context(
                tc.tile_pool(name="dram", bufs=1, space=bass.MemorySpace.DRAM))

            I = const_pool.tile([P, P], F32, name="I")
            nc.sync.dma_start(I[:], id_dram.ap())
            ones_bf = const_pool.tile([P, 1], BF16, name="ones_bf")
            nc.vector.memset(ones_bf[:], 1.0)
            gam_sb = const_pool.tile([P, 2, 10], F32, name="gam_sb")
            beta_sb = const_pool.tile([P, 2, NCOMP], F32, name="beta_sb")
            for h in range(2):
                nc.sync.dma_start(gam_sb[:, h, :], gam_dram.ap()[h * P:(h + 1) * P, :])
                nc.sync.dma_start(beta_sb[:, h, :], beta_dram.ap()[h * P:(h + 1) * P, :])

            # ---------------- Phase 1: stats ----------------
            # pools stay open (outer stack) so phase-2 allocations get
            # disjoint addresses -> scheduler can prefetch phase-2 loads
            # during phase 1 / the allreduce+whitening bubble
            with (
                tc.tile_pool(name="ph1_psum", bufs=1, space=bass.MemorySpace.PSUM) as pp,
                tc.tile_pool(name="ph1_sbuf", bufs=1) as p1s,
                tc.tile_pool(name="xbf_pool", bufs=1) as xbf_pool,
            ):
                # 20 gram accumulators [128,128] packed 4-per-bank; means [128,8]
                gbank = [pp.tile([P, 512], F32, name=f"gbank{i}") for i in range(5)]
                mbank = pp.tile([P, 8], F32, name="mbank")

                def gslot(t, h):
                    idx = t * 2 + h
                    b, c0 = idx // 4, (idx % 4) * P
                    return gbank[b][:, c0:c0 + P]

                # PSUM start=True zeroes the whole 2KB bank (pending-zero
                # granularity), so emit exactly one start (and one stop) per
                # bank: on the first/last matmul touching it in the fixed
                # (h, p, q) emission order.
                seq = []           # (kind, bank_key)
                for h in range(2):
                    for p in range(NCOMP):
                        seq.append("mbank")
                        for q in range(p, NCOMP):
                            seq.append((TRI_IDX[(p, q)] * 2 + h) // 4)
                first_touch = {}
                last_touch = {}
                for i, b in enumerate(seq):
                    if b not in first_touch:
                        first_touch[b] = i
                    last_touch[b] = i

                for ci, (s0, rows) in enumerate(chunks):
                    nblk = rows // P
                    xbf = []
                    for p in range(NCOMP):
                        t_ = xbf_pool.tile([P, nblk, C], BF16,
                                           name=f"xbf{p}", tag=f"xbf{p}", bufs=2)
                        # partition owns `nblk` consecutive rows (contiguous
                        # nblk*1KB per partition -> efficient SWDGE cast-DMA)
                        src = x_dram.ap()[p, s0:s0 + rows, :].rearrange(
                            "(p m) c -> p m c", p=P)
                        nc.gpsimd.dma_start(t_[:], src)
                        xbf.append(t_)
                    first = ci == 0
                    last = ci == last_ci
                    for m in range(nblk):
                        st_first = first and m == 0
                        st_last = last and m == nblk - 1
                        si = 0
                        for h in range(2):
                            for p in range(NCOMP):
                                st = xbf[p][:, m, h * P:(h + 1) * P]
                                nc.tensor.matmul(
                                    mbank[:, p * 2 + h:p * 2 + h + 1], st, ones_bf[:],
                                    start=st_first and first_touch[seq[si]] == si,
                                    stop=st_last and last_touch[seq[si]] == si,
                                    skip_group_check=True)
                                si += 1
                                for q in range(p, NCOMP):
                                    nc.tensor.matmul(
                                        gslot(TRI_IDX[(p, q)], h), st,
                                        xbf[q][:, m, h * P:(h + 1) * P],
                                        start=st_first and first_touch[seq[si]] == si,
                                        stop=st_last and last_touch[seq[si]] == si,
                                        skip_group_check=True)
                                    si += 1

                # drain stats -> [128, 14, 2] (items: 4 means, 10 gram diags)
                stats_sb = p1s.tile([P, 14, 2], F32, name="stats_sb")
                for p in range(NCOMP):
                    for h in range(2):
                        nc.scalar.copy(stats_sb[:, p, h:h + 1],
                                       mbank[:, p * 2 + h:p * 2 + h + 1])
                for t in range(10):
                    for h in range(2):
                        masked = p1s.tile([P, P], F32, name="masked",
                                          tag="masked", bufs=2)
                        nc.vector.tensor_mul(masked[:], gslot(t, h), I[:])
                        nc.vector.tensor_reduce(
                            out=stats_sb[:, 4 + t, h:h + 1], in_=masked[:],
                            axis=mybir.AxisListType.X, op=AOP.add)

                # AllReduce partial sums across cores
                part_dram = dram_pool.tile([P, 28], F32, name="part_dram")
                cc_dram = dram_pool.tile([P, 28], F32, name="cc_dram",
                                         addr_space="Shared" if n_cores > 4 else "Local")
                nc.scalar.dma_start(part_dram[:], stats_sb[:].rearrange("p a b -> p (a b)"))
                if n_cores > 1:
                    nc.gpsimd.collective_compute(
                        "AllReduce", AOP.add,
                        replica_groups=[list(range(n_cores))],
                        ins=[part_dram.opt()], outs=[cc_dram.opt()])
                    src_stats = cc_dram
                else:
                    src_stats = part_dram
                stats_g = wh_pool.tile([P, 14, 2], F32, name="stats_g")
                nc.scalar.dma_start(stats_g[:].rearrange("p a b -> p (a b)"), src_stats[:])

            # ---------------- whitening math on [128,2] tiles ----------------
            def wt(name):
                return wh_pool.tile([P, 2], F32, name=name, tag=name)

            def vmul(o, a, b):
                nc.vector.tensor_mul(o[:], a[:], b[:])

            def vadd(o, a, b):
                nc.vector.tensor_add(o[:], a[:], b[:])

            def vsub(o, a, b):
                nc.vector.tensor_tensor(o[:], a[:], b[:], AOP.subtract)

            def recip(name, a):
                o = wt(name)
                nc.vector.reciprocal(o[:], a[:])
                return o

            def sqrt_nr(name, v):
                s0 = wt(name + "_s0")
                nc.scalar.sqrt(s0[:], v[:])
                r = recip(name + "_r", s0)
                q = wt(name + "_q")
                vmul(q, v, r)
                s = wt(name + "_s")
                vadd(s, s0, q)
                o = wt(name)
                nc.vector.tensor_scalar_mul(o[:], s[:], 0.5)
                return o

            mu = []
            for p in range(NCOMP):
                m_ = wt(f"mu{p}")
                nc.vector.tensor_scalar_mul(m_[:], stats_g[:, p, :], 1.0 / NTOT)
                mu.append(m_)

            v = {}
            for t, (p, q) in enumerate(TRI):
                name = NAMES[p] + NAMES[q]
                mm = wt(f"mm_{name}")
                vmul(mm, mu[p], mu[q])
                if p == q:
                    nc.vector.tensor_scalar_add(mm[:], mm[:], -EPS)
                vv = wt(f"v_{name}")
                # vv = G/NTOT - (mu_p mu_q - eps_diag)
                nc.vector.scalar_tensor_tensor(
                    out=vv[:], in0=stats_g[:, 4 + t, :], scalar=1.0 / NTOT,
                    in1=mm[:], op0=AOP.mult, op1=AOP.subtract)
                v[name] = vv

            w = {}
            w['rr'] = sqrt_nr("w_rr", v['rr'])
            rc_rr = recip("rc_rr", w['rr'])
            for nm in ('ri', 'rj', 'rk'):
                w[nm] = wt(f"w_{nm}")
                vmul(w[nm], v[nm], rc_rr)
            t1 = wt("t_ii")
            vmul(t1, w['ri'], w['ri'])
            t2 = wt("t_ii2")
            vsub(t2, v['ii'], t1)
            w['ii'] = sqrt_nr("w_ii", t2)
            rc_ii = recip("rc_ii", w['ii'])
            for nm, a, b in (("ij", 'ri', 'rj'), ("ik", 'ri', 'rk')):
                u1 = wt(f"u_{nm}")
                vmul(u1, w[a], w[b])
                u2 = wt(f"u2_{nm}")
                vsub(u2, v[nm], u1)
                w[nm] = wt(f"w_{nm}")
                vmul(w[nm], u2, rc_ii)
            u3 = wt("u_jj")
            vmul(u3, w['ij'], w['ij'])
            u4 = wt("u_jj2")
            vmul(u4, w['rj'], w['rj'])
            u5 = wt("u_jj3")
            vadd(u5, u3, u4)
            u6 = wt("u_jj4")
            vsub(u6, v['jj'], u5)
            w['jj'] = sqrt_nr("w_jj", u6)
            rc_jj = recip("rc_jj", w['jj'])
            u7 = wt("u_jk")
            vmul(u7, w['ij'], w['ik'])
            u8 = wt("u_jk2")
            vmul(u8, w['rj'], w['rk'])
            u9 = wt("u_jk3")
            vadd(u9, u7, u8)
            u10 = wt("u_jk4")
            vsub(u10, v['jk'], u9)
            w['jk'] = wt("w_jk")
            vmul(w['jk'], u10, rc_jj)
            u11 = wt("u_kk")
            vmul(u11, w['jk'], w['jk'])
            u12 = wt("u_kk2")
            vmul(u12, w['ik'], w['ik'])
            u13 = wt("u_kk3")
            vadd(u13, u11, u12)
            u14 = wt("u_kk4")
            vmul(u14, w['rk'], w['rk'])
            u15 = wt("u_kk5")
            vadd(u15, u13, u14)
            u16 = wt("u_kk6")
            vsub(u16, v['kk'], u15)
            w['kk'] = sqrt_nr("w_kk", u16)
            rc_kk = recip("rc_kk", w['kk'])

            o = {}
            o['rr'], o['ii'], o['jj'], o['kk'] = rc_rr, rc_ii, rc_jj, rc_kk

            def neg_mul(name, a, b, rc):
                # returns -(a*b)*rc
                z1 = wt(name + "_z1")
                vmul(z1, a, b)
                z2 = wt(name + "_z2")
                vmul(z2, z1, rc)
                z3 = wt(name)
                nc.vector.tensor_scalar_mul(z3[:], z2[:], -1.0)
                return z3

            o['ri'] = neg_mul("o_ri", w['ri'], o['rr'], rc_ii)
            z1 = wt("ork_a")
            vmul(z1, w['rj'], o['rr'])
            z2 = wt("ork_b")
            vmul(z2, w['ij'], o['ri'])
            z3 = wt("ork_c")
            vadd(z3, z1, z2)
            z4 = wt("ork_d")
            vmul(z4, z3, rc_jj)
            o['rj'] = wt("o_rj")
            nc.vector.tensor_scalar_mul(o['rj'][:], z4[:], -1.0)
            y1 = wt("orkk_a")
            vmul(y1, w['rk'], o['rr'])
            y2 = wt("orkk_b")
            vmul(y2, w['ik'], o['ri'])
            y3 = wt("orkk_c")
            vmul(y3, w['jk'], o['rj'])
            y4 = wt("orkk_d")
            vadd(y4, y1, y2)
            y5 = wt("orkk_e")
            vadd(y5, y4, y3)
            y6 = wt("orkk_f")
            vmul(y6, y5, rc_kk)
            o['rk'] = wt("o_rk")
            nc.vector.tensor_scalar_mul(o['rk'][:], y6[:], -1.0)
            o['ij'] = neg_mul("o_ij", w['ij'], o['ii'], rc_jj)
            x1 = wt("oik_a")
            vmul(x1, w['ik'], o['ii'])
            x2 = wt("oik_b")
            vmul(x2, w['jk'], o['ij'])
            x3 = wt("oik_c")
            vadd(x3, x1, x2)
            x4 = wt("oik_d")
            vmul(x4, x3, rc_kk)
            o['ik'] = wt("o_ik")
            nc.vector.tensor_scalar_mul(o['ik'][:], x4[:], -1.0)
            o['jk'] = neg_mul("o_jk", w['jk'], o['jj'], rc_kk)

            def Wsym(a, b):
                i1, i2 = min(a, b), max(a, b)
                return o[NAMES[i1] + NAMES[i2]]

            def Gsym(a, b):
                return gam_sb[:, :, TRI_IDX[(a, b)]]

            # M[p][q] = sum_s G(p,s) W(s,q); bprime[p] = beta_p - sum_q M[p][q] mu_q
            Mt = [[None] * NCOMP for _ in range(NCOMP)]
            bp = [None] * NCOMP
            for p in range(NCOMP):
                for q in range(NCOMP):
                    acc = wh_pool.tile([P, 2], F32, name=f"M{p}{q}", tag=f"M{p}{q}")
                    tmp0 = wt(f"Mt{p}{q}_0")
                    vmul(tmp0, Gsym(p, 0), Wsym(0, q))
                    nc.vector.tensor_copy(acc[:], tmp0[:])
                    for s_ in range(1, NCOMP):
                        tmp = wt(f"Mt{p}{q}_{s_}")
                        vmul(tmp, Gsym(p, s_), Wsym(s_, q))
                        vadd(acc, acc, tmp)
                    Mt[p][q] = acc
                bacc_t = wh_pool.tile([P, 2], F32, name=f"bp{p}", tag=f"bp{p}")
                nc.vector.tensor_copy(bacc_t[:], beta_sb[:, :, p])
                for q in range(NCOMP):
                    tmp = wt(f"bp{p}_{q}")
                    vmul(tmp, Mt[p][q], mu[q])
                    vsub(bacc_t, bacc_t, tmp)
                bp[p] = bacc_t

            if debug_out:
                nc.sync.dma_start(dbg_stats.ap(), stats_g[:].rearrange("p a b -> p (a b)"))
                for p_ in range(NCOMP):
                    for q_ in range(NCOMP):
                        nc.sync.dma_start(dbg_m.ap()[:, :, p_ * 4 + q_], Mt[p_][q_][:])
                    nc.sync.dma_start(dbg_bp.ap()[:, :, p_], bp[p_][:])

            # ---------------- Phase 2: apply ----------------
            with (
                tc.tile_pool(name="xf_pool", bufs=1) as xf_pool,
                tc.tile_pool(name="strip_psum", bufs=1, space=bass.MemorySpace.PSUM) as sp,
                tc.tile_pool(name="xT_pool", bufs=1) as xT_pool,
                tc.tile_pool(name="chain_pool", bufs=1) as chain_pool,
                tc.tile_pool(name="out_pool", bufs=1) as out_pool,
            ):
                for ci, (s0, rows) in enumerate(chunks):
                    nblk = rows // P
                    xf = []
                    for p in range(NCOMP):
                        t_ = xf_pool.tile([P, nblk, C], F32,
                                          name=f"xf{p}", tag=f"xf{p}", bufs=2)
                        # partition = row within each 128-block (transposable)
                        src = x_dram.ap()[p, s0:s0 + rows, :].rearrange(
                            "(m p) c -> p m c", p=P)
                        nc.sync.dma_start(t_[:], src)
                        xf.append(t_)
                    for h in range(2):
                        xT = []
                        for p in range(NCOMP):
                            xt = xT_pool.tile([P, rows], F32,
                                              name=f"xT{p}", tag=f"xT{p}", bufs=2)
                            m0 = 0
                            while m0 < nblk:
                                g = min(4, nblk - m0)
                                wdt = g * P
                                strip = sp.tile([P, 512], F32, name="strip",
                                                tag="strip", bufs=6)
                                for ji in range(g):
                                    nc.tensor.transpose(
                                        strip[:, ji * P:(ji + 1) * P],
                                        xf[p][:, m0 + ji, h * P:(h + 1) * P], I[:])
                                nc.scalar.copy(xt[:, m0 * P:m0 * P + wdt],
                                               strip[:, 0:wdt])
                                m0 += g
                            xT.append(xt)
                        for q in range(NCOMP):
                            c0 = chain_pool.tile([P, rows], F32, name="chain0",
                                                 tag="chain", bufs=6)
                            nc.scalar.activation(
                                c0[:], xT[0][:], AF.Identity,
                                scale=Mt[q][0][:, h:h + 1], bias=bp[q][:, h:h + 1])
                            c1 = chain_pool.tile([P, rows], F32, name="chain1",
                                                 tag="chain", bufs=6)
                            nc.vector.scalar_tensor_tensor(
                                out=c1[:], in0=xT[1][:], scalar=Mt[q][1][:, h:h + 1],
                                in1=c0[:], op0=AOP.mult, op1=AOP.add)
                            c2 = chain_pool.tile([P, rows], F32, name="chain2",
                                                 tag="chain", bufs=6)
                            nc.vector.scalar_tensor_tensor(
                                out=c2[:], in0=xT[2][:], scalar=Mt[q][2][:, h:h + 1],
                                in1=c1[:], op0=AOP.mult, op1=AOP.add)
                            oq = out_pool.tile([P, rows], F32, name="oq",
                                               tag="oq", bufs=4)
                            nc.vector.scalar_tensor_tensor(
                                out=oq[:], in0=xT[3][:], scalar=Mt[q][3][:, h:h + 1],
                                in1=c2[:], op0=AOP.mult, op1=AOP.add)
                            nc.sync.dma_start(
                                out_dram.ap()[q, h * P:(h + 1) * P, s0:s0 + rows],
                                oq[:])

    nc.compile()
    return nc


_BUILD_CACHE = {}


def _get_bass(S, n_cores):
    key = (S, n_cores)
    if key not in _BUILD_CACHE:
        _BUILD_CACHE[key] = build_bass(S, n_cores)
    return _BUILD_CACHE[key]


def _run(x, gamma, beta, trace=False):
    x = np.asarray(x)
    gamma = np.asarray(gamma)
    beta = np.asarray(beta)
    n_cores = 8
    four, B, H, W, Cc = x.shape
    bpc = B // n_cores           # batches per core
    S = bpc * H * W

    gam_t = np.ascontiguousarray(gamma.T.astype(np.float32))
    beta_t = np.ascontiguousarray(beta.T.astype(np.float32))
    ident = np.eye(P, dtype=np.float32)

    in_maps = []
    for k in range(n_cores):
        shard = np.ascontiguousarray(
            x[:, k * bpc:(k + 1) * bpc].reshape(four, S, Cc))
        in_maps.append({"x": shard, "gammaT": gam_t, "betaT": beta_t,
                        "ident": ident})

    nc = _get_bass(S, n_cores)
    res = run_bass_kernel_spmd(nc, in_maps, list(range(n_cores)), trace=trace)

    out = np.empty((four, B, H, W, Cc), dtype=np.float32)
    for k in range(n_cores):
        o = res.results[k]["out_t"]          # [4, C, S]
        out[:, k * bpc:(k + 1) * bpc] = (
            o.transpose(0, 2, 1).reshape(four, bpc, H, W, Cc))
    return out, res


def kernel(x, gamma, beta):
    """x [4,32,56,56,256] f32; gamma [10,256]; beta [4,256] -> [4,32,56,56,256]."""
    out, _ = _run(x, gamma, beta)
    return out


# revision 14
# speedup vs baseline: 159.3230x; 159.3230x over previous
"""Quaternion batch-norm (nn_BatchNormalizationQ) Trainium2 kernel.

Strategy (8 NeuronCores, batch-parallel):
  - Host shards x [4,32,56,56,256] on batch -> per core [4, 12544, 256]
    (component, spatial, channel); gamma/beta pre-transposed to [256,*].
  - Phase 1 (stats): cast-DMA tiles to bf16, PE computes per-channel
    Gram sums  sum_s x_p x_q  (10 pairs) and sums  sum_s x_p  (via
    ones-vector matmuls), accumulated in PSUM.  Diagonals extracted with
    identity-mask multiply + row-reduce.  Partial sums [128,28] are
    AllReduced across the 8 cores.
  - Whitening: per-channel 4x4 inverse-Cholesky + gamma fusion computed
    on-chip on [128,2] tiles (channel on partitions), giving M[4][4] and
    b' = beta - M mu as per-partition scalar vectors.
  - Phase 2 (apply): fp32 tiles, PE-transposed to channel-major strips in
    PSUM, drained to SBUF by ACT; out_q = ((x0*M_q0+b'_q) chained with
    scalar_tensor_tensor FMAs on DVE.  Output written channel-major
    [4, 256, 12544] per core; host transposes back.
"""
import numpy as np

from concourse import bass, bacc, tile, mybir
from concourse.bass_utils import run_bass_kernel_spmd

F32 = mybir.dt.float32
BF16 = mybir.dt.bfloat16
AOP = mybir.AluOpType
AF = mybir.ActivationFunctionType

P = 128
C = 256          # channels
NCOMP = 4        # quaternion components
EPS = 1e-4
CHUNK = 1024     # spatial rows per chunk (must be multiple of 128)

NAMES = "rijk"
TRI = [(p1, p2) for p1 in range(4) for p2 in range(p1, 4)]
TRI_IDX = {}
for _i, (_p, _q) in enumerate(TRI):
    TRI_IDX[(_p, _q)] = _i
    TRI_IDX[(_q, _p)] = _i


def _chunks(S):
    out = []
    s0 = 0
    while s0 < S:
        rows = min(CHUNK, S - s0)
        out.append((s0, rows))
        s0 += rows
    return out


def build_bass(S, n_cores, debug_out=False):
    """Build the SPMD program for per-core spatial size S over n_cores."""
    NTOT = float(S * n_cores)
    nc = bacc.Bacc("TRN2", target_bir_lowering=False, debug=False,
                   num_devices=n_cores)

    x_dram = nc.dram_tensor("x", [NCOMP, S, C], F32, kind="ExternalInput")
    gam_dram = nc.dram_tensor("gammaT", [C, 10], F32, kind="ExternalInput")
    beta_dram = nc.dram_tensor("betaT", [C, NCOMP], F32, kind="ExternalInput")
    id_dram = nc.dram_tensor("ident", [P, P], F32, kind="ExternalInput")
    out_dram = nc.dram_tensor("out_t", [NCOMP, C, S], F32, kind="ExternalOutput")
    if debug_out:
        dbg_stats = nc.dram_tensor("dbg_stats", [P, 28], F32, kind="ExternalOutput")
        dbg_m = nc.dram_tensor("dbg_m", [P, 2, 16], F32, kind="ExternalOutput")
        dbg_bp = nc.dram_tensor("dbg_bp", [P, 2, 4], F32, kind="ExternalOutput")

    chunks = _chunks(S)
    last_ci = len(chunks) - 1

    with tile.TileContext(nc) as tc:
        import contextlib
        stack = contextlib.ExitStack()
        with stack:
            const_pool = stack.enter_context(tc.tile_pool(name="consts", bufs=1))
            wh_pool = stack.enter_context(tc.tile_pool(name="whiten", bufs=1))
            dram_pool = stack.enter_context(
                tc.tile_pool(name="dram", bufs=1, space=bass.MemorySpace.DRAM))

            I = const_pool.tile([P, P], F32, name="I")
            nc.sync.dma_start(I[:], id_dram.ap())
            ones_bf = const_pool.tile([P, 1], BF16, name="ones_bf")
            nc.vector.memset(ones_bf[:], 1.0)
            gam_sb = const_pool.tile([P, 2, 10], F32, name="gam_sb")
            beta_sb = const_pool.tile([P, 2, NCOMP], F32, name="beta_sb")
            for h in range(2):
                nc.sync.dma_start(gam_sb[:, h, :], gam_dram.ap()[h * P:(h + 1) * P, :])
                nc.sync.dma_start(beta_sb[:, h, :], beta_dram.ap()[h * P:(h + 1) * P, :])

            # ---------------- Phase 1: stats ----------------
            # pools stay open (outer stack) so phase-2 allocations get
            # disjoint addresses -> scheduler can prefetch phase-2 loads
            # during phase 1 / the allreduce+whitening bubble
            with (
                tc.tile_pool(name="ph1_psum", bufs=1, space=bass.MemorySpace.PSUM) as pp,
                tc.tile_pool(name="ph1_sbuf", bufs=1) as p1s,
                tc.tile_pool(name="xbf_pool", bufs=1) as xbf_pool,
            ):
                # 20 gram accumulators [128,128] packed 4-per-bank; means [128,8]
                gbank = [pp.tile([P, 512], F32, name=f"gbank{i}") for i in range(5)]
                mbank = pp.tile([P, 8], F32, name="mbank")

                def gslot(t, h):
                    idx = t * 2 + h
                    b, c0 = idx // 4, (idx % 4) * P
                    return gbank[b][:, c0:c0 + P]

                # PSUM start=True zeroes the whole 2KB bank (pending-zero
                # granularity), so emit exactly one start (and one stop) per
                # bank: on the first/last matmul touching it in the fixed
                # (h, p, q) emission order.
                seq = []           # (kind, bank_key)
                for h in range(2):
                    for p in range(NCOMP):
                        seq.append("mbank")
                        for q in range(p, NCOMP):
                            seq.append((TRI_IDX[(p, q)] * 2 + h) // 4)
                first_touch = {}
                last_touch = {}
                for i, b in enumerate(seq):
                    if b not in first_touch:
                        first_touch[b] = i
                    last_touch[b] = i

                for ci, (s0, rows) in enumerate(chunks):
                    nblk = rows // P
                    xbf = []
                    for p in range(NCOMP):
                        t_ = xbf_pool.tile([P, nblk, C], BF16,
                                           name=f"xbf{p}", tag=f"xbf{p}", bufs=2)
                        # partition owns `nblk` consecutive rows (contiguous
                        # nblk*1KB per partition -> efficient SWDGE cast-DMA)
                        src = x_dram.ap()[p, s0:s0 + rows, :].rearrange(
                            "(p m) c -> p m c", p=P)
                        nc.gpsimd.dma_start(t_[:], src)
                        xbf.append(t_)
                    first = ci == 0
                    last = ci == last_ci
                    for m in range(nblk):
                        st_first = first and m == 0
                        st_last = last and m == nblk - 1
                        si = 0
                        for h in range(2):
                            for p in range(NCOMP):
                                st = xbf[p][:, m, h * P:(h + 1) * P]
                                nc.tensor.matmul(
                                    mbank[:, p * 2 + h:p * 2 + h + 1], st, ones_bf[:],
                                    start=st_first and first_touch[seq[si]] == si,
                                    stop=st_last and last_touch[seq[si]] == si,
                                    skip_group_check=True)
                                si += 1
                                for q in range(p, NCOMP):
                                    nc.tensor.matmul(
                                        gslot(TRI_IDX[(p, q)], h), st,
                                        xbf[q][:, m, h * P:(h + 1) * P],
                                        start=st_first and first_touch[seq[si]] == si,
                                        stop=st_last and last_touch[seq[si]] == si,
                                        skip_group_check=True)
                                    si += 1

                # drain stats -> [128, 14, 2] (items: 4 means, 10 gram diags)
                stats_sb = p1s.tile([P, 14, 2], F32, name="stats_sb")
                for p in range(NCOMP):
                    for h in range(2):
                        nc.scalar.copy(stats_sb[:, p, h:h + 1],
                                       mbank[:, p * 2 + h:p * 2 + h + 1])
                for t in range(10):
                    for h in range(2):
                        masked = p1s.tile([P, P], F32, name="masked",
                                          tag="masked", bufs=2)
                        nc.vector.tensor_mul(masked[:], gslot(t, h), I[:])
                        nc.vector.tensor_reduce(
                            out=stats_sb[:, 4 + t, h:h + 1], in_=masked[:],
                            axis=mybir.AxisListType.X, op=AOP.add)

                # AllReduce partial sums across cores
                part_dram = dram_pool.tile([P, 28], F32, name="part_dram")
                cc_dram = dram_pool.tile([P, 28], F32, name="cc_dram",
                                         addr_space="Shared" if n_cores > 4 else "Local")
                nc.scalar.dma_start(part_dram[:], stats_sb[:].rearrange("p a b -> p (a b)"))
                if n_cores > 1:
                    nc.gpsimd.collective_compute(
                        "AllReduce", AOP.add,
                        replica_groups=[list(range(n_cores))],
                        ins=[part_dram.opt()], outs=[cc_dram.opt()])
                    src_stats = cc_dram
                else:
                    src_stats = part_dram
                stats_g = wh_pool.tile([P, 14, 2], F32, name="stats_g")
                nc.scalar.dma_start(stats_g[:].rearrange("p a b -> p (a b)"), src_stats[:])

            # ---------------- whitening math on [128,2] tiles ----------------
            def wt(name):
                return wh_pool.tile([P, 2], F32, name=name, tag=name)

            def vmul(o, a, b):
                nc.vector.tensor_mul(o[:], a[:], b[:])

            def vadd(o, a, b):
                nc.vector.tensor_add(o[:], a[:], b[:])

            def vsub(o, a, b):
                nc.vector.tensor_tensor(o[:], a[:], b[:], AOP.subtract)

            def recip(name, a):
                o = wt(name)
                nc.vector.reciprocal(o[:], a[:])
                return o

            def sqrt_nr(name, v):
                s0 = wt(name + "_s0")
                nc.scalar.sqrt(s0[:], v[:])
                r = recip(name + "_r", s0)
                q = wt(name + "_q")
                vmul(q, v, r)
                s = wt(name + "_s")
                vadd(s, s0, q)
                o = wt(name)
                nc.vector.tensor_scalar_mul(o[:], s[:], 0.5)
                return o

            mu = []
            for p in range(NCOMP):
                m_ = wt(f"mu{p}")
                nc.vector.tensor_scalar_mul(m_[:], stats_g[:, p, :], 1.0 / NTOT)
                mu.append(m_)

            v = {}
            for t, (p, q) in enumerate(TRI):
                name = NAMES[p] + NAMES[q]
                mm = wt(f"mm_{name}")
                vmul(mm, mu[p], mu[q])
                if p == q:
                    nc.vector.tensor_scalar_add(mm[:], mm[:], -EPS)
                vv = wt(f"v_{name}")
                # vv = G/NTOT - (mu_p mu_q - eps_diag)
                nc.vector.scalar_tensor_tensor(
                    out=vv[:], in0=stats_g[:, 4 + t, :], scalar=1.0 / NTOT,
                    in1=mm[:], op0=AOP.mult, op1=AOP.subtract)
                v[name] = vv

            w = {}
            w['rr'] = sqrt_nr("w_rr", v['rr'])
            rc_rr = recip("rc_rr", w['rr'])
            for nm in ('ri', 'rj', 'rk'):
                w[nm] = wt(f"w_{nm}")
                vmul(w[nm], v[nm], rc_rr)
            t1 = wt("t_ii")
            vmul(t1, w['ri'], w['ri'])
            t2 = wt("t_ii2")
            vsub(t2, v['ii'], t1)
            w['ii'] = sqrt_nr("w_ii", t2)
            rc_ii = recip("rc_ii", w['ii'])
            for nm, a, b in (("ij", 'ri', 'rj'), ("ik", 'ri', 'rk')):
                u1 = wt(f"u_{nm}")
                vmul(u1, w[a], w[b])
                u2 = wt(f"u2_{nm}")
                vsub(u2, v[nm], u1)
                w[nm] = wt(f"w_{nm}")
                vmul(w[nm], u2, rc_ii)
            u3 = wt("u_jj")
            vmul(u3, w['ij'], w['ij'])
            u4 = wt("u_jj2")
            vmul(u4, w['rj'], w['rj'])
            u5 = wt("u_jj3")
            vadd(u5, u3, u4)
            u6 = wt("u_jj4")
            vsub(u6, v['jj'], u5)
            w['jj'] = sqrt_nr("w_jj", u6)
            rc_jj = recip("rc_jj", w['jj'])
            u7 = wt("u_jk")
            vmul(u7, w['ij'], w['ik'])
            u8 = wt("u_jk2")
            vmul(u8, w['rj'], w['rk'])
            u9 = wt("u_jk3")
            vadd(u9, u7, u8)
            u10 = wt("u_jk4")
            vsub(u10, v['jk'], u9)
            w['jk'] = wt("w_jk")
            vmul(w['jk'], u10, rc_jj)
            u11 = wt("u_kk")
            vmul(u11, w['jk'], w['jk'])
            u12 = wt("u_kk2")
            vmul(u12, w['ik'], w['ik'])
            u13 = wt("u_kk3")
            vadd(u13, u11, u12)
            u14 = wt("u_kk4")
            vmul(u14, w['rk'], w['rk'])
            u15 = wt("u_kk5")
            vadd(u15, u13, u14)
            u16 = wt("u_kk6")
            vsub(u16, v['kk'], u15)
            w['kk'] = sqrt_nr("w_kk", u16)
            rc_kk = recip("rc_kk", w['kk'])

            o = {}
            o['rr'], o['ii'], o['jj'], o['kk'] = rc_rr, rc_ii, rc_jj, rc_kk

            def neg_mul(name, a, b, rc):
                # returns -(a*b)*rc
                z1 = wt(name + "_z1")
                vmul(z1, a, b)
                z2 = wt(name + "_z2")
                vmul(z2, z1, rc)
                z3 = wt(name)
                nc.vector.tensor_scalar_mul(z3[:], z2[:], -1.0)
                return z3

            o['ri'] = neg_mul("o_ri", w['ri'], o['rr'], rc_ii)
            z1 = wt("ork_a")
            vmul(z1, w['rj'], o['rr'])
            z2 = wt("ork_b")
            vmul(z2, w['ij'], o['ri'])
            z3 = wt("ork_c")
            vadd(z3, z1, z2)
            z4 = wt("ork_d")
            vmul(z4, z3, rc_jj)
            o['rj'] = wt("o_rj")
            nc.vector.tensor_scalar_mul(o['rj'][:], z4[:], -1.0)
            y1 = wt("orkk_a")
            vmul(y1, w['rk'], o['rr'])
            y2 = wt("orkk_b")
            vmul(y2, w['ik'], o['ri'])
            y3 = wt("orkk_c")
            vmul(y3, w['jk'], o['rj'])
            y4 = wt("orkk_d")
            vadd(y4, y1, y2)
            y5 = wt("orkk_e")
            vadd(y5, y4, y3)
            y6 = wt("orkk_f")
            vmul(y6, y5, rc_kk)
            o['rk'] = wt("o_rk")
            nc.vector.tensor_scalar_mul(o['rk'][:], y6[:], -1.0)
            o['ij'] = neg_mul("o_ij", w['ij'], o['ii'], rc_jj)
            x1 = wt("oik_a")
            vmul(x1, w['ik'], o['ii'])
            x2 = wt("oik_b")
            vmul(x2, w['jk'], o['ij'])
            x3 = wt("oik_c")
            vadd(x3, x1, x2)
            x4 = wt("oik_d")
            vmul(x4, x3, rc_kk)
            o['ik'] = wt("o_ik")
            nc.vector.tensor_scalar_mul(o['ik'][:], x4[:], -1.0)
            o['jk'] = neg_mul("o_jk", w['jk'], o['jj'], rc_kk)

            def Wsym(a, b):
                i1, i2 = min(a, b), max(a, b)
                return o[NAMES[i1] + NAMES[i2]]

            def Gsym(a, b):
                return gam_sb[:, :, TRI_IDX[(a, b)]]

            # M[p][q] = sum_s G(p,s) W(s,q); bprime[p] = beta_p - sum_q M[p][q] mu_q
            Mt = [[None] * NCOMP for _ in range(NCOMP)]
            bp = [None] * NCOMP
            for p in range(NCOMP):
                for q in range(NCOMP):
                    acc = wh_pool.tile([P, 2], F32, name=f"M{p}{q}", tag=f"M{p}{q}")
                    tmp0 = wt(f"Mt{p}{q}_0")
                    vmul(tmp0, Gsym(p, 0), Wsym(0, q))
                    nc.vector.tensor_copy(acc[:], tmp0[:])
                    for s_ in range(1, NCOMP):
                        tmp = wt(f"Mt{p}{q}_{s_}")
                        vmul(tmp, Gsym(p, s_), Wsym(s_, q))
                        vadd(acc, acc, tmp)
                    Mt[p][q] = acc
                bacc_t = wh_pool.tile([P, 2], F32, name=f"bp{p}", tag=f"bp{p}")
                nc.vector.tensor_copy(bacc_t[:], beta_sb[:, :, p])
                for q in range(NCOMP):
                    tmp = wt(f"bp{p}_{q}")
                    vmul(tmp, Mt[p][q], mu[q])
                    vsub(bacc_t, bacc_t, tmp)
                bp[p] = bacc_t

            if debug_out:
                nc.sync.dma_start(dbg_stats.ap(), stats_g[:].rearrange("p a b -> p (a b)"))
                for p_ in range(NCOMP):
                    for q_ in range(NCOMP):
                        nc.sync.dma_start(dbg_m.ap()[:, :, p_ * 4 + q_], Mt[p_][q_][:])
                    nc.sync.dma_start(dbg_bp.ap()[:, :, p_], bp[p_][:])

            # ---------------- Phase 2: apply ----------------
            with (
                tc.tile_pool(name="xf_pool", bufs=1) as xf_pool,
                tc.tile_pool(name="strip_psum", bufs=1, space=bass.MemorySpace.PSUM) as sp,
                tc.tile_pool(name="xT_pool", bufs=1) as xT_pool,
                tc.tile_pool(name="chain_pool", bufs=1) as chain_pool,
                tc.tile_pool(name="out_pool", bufs=1) as out_pool,
            ):
                for ci, (s0, rows) in enumerate(chunks):
                    nblk = rows // P
                    xf = []
                    for p in range(NCOMP):
                        t_ = xf_pool.tile([P, nblk, C], F32,
                                          name=f"xf{p}", tag=f"xf{p}", bufs=2)
                        # partition = row within each 128-block (transposable)
                        src = x_dram.ap()[p, s0:s0 + rows, :].rearrange(
                            "(m p) c -> p m c", p=P)
                        nc.sync.dma_start(t_[:], src)
                        xf.append(t_)
                    for h in range(2):
                        xT = []
                        for p in range(NCOMP):
                            xt = xT_pool.tile([P, rows], F32,
                                              name=f"xT{p}", tag=f"xT{p}", bufs=2)
                            m0 = 0
                            while m0 < nblk:
                                g = min(4, nblk - m0)
                                wdt = g * P
                                strip = sp.tile([P, 512], F32, name="strip",
                                                tag="strip", bufs=6)
                                for ji in range(g):
                                    nc.tensor.transpose(
                                        strip[:, ji * P:(ji + 1) * P],
                                        xf[p][:, m0 + ji, h * P:(h + 1) * P], I[:])
                                nc.scalar.copy(xt[:, m0 * P:m0 * P + wdt],
                                               strip[:, 0:wdt])
                                m0 += g
                            xT.append(xt)
                        for q in range(NCOMP):
                            c0 = chain_pool.tile([P, rows], F32, name="chain0",
                                                 tag="chain", bufs=6)
                            nc.scalar.activation(
                                c0[:], xT[0][:], AF.Identity,
                                scale=Mt[q][0][:, h:h + 1], bias=bp[q][:, h:h + 1])
                            c1 = chain_pool.tile([P, rows], F32, name="chain1",
                                                 tag="chain", bufs=6)
                            nc.vector.scalar_tensor_tensor(
                                out=c1[:], in0=xT[1][:], scalar=Mt[q][1][:, h:h + 1],
                                in1=c0[:], op0=AOP.mult, op1=AOP.add)
                            c2 = chain_pool.tile([P, rows], F32, name="chain2",
                                                 tag="chain", bufs=6)
                            nc.vector.scalar_tensor_tensor(
                                out=c2[:], in0=xT[2][:], scalar=Mt[q][2][:, h:h + 1],
                                in1=c1[:], op0=AOP.mult, op1=AOP.add)
                            oq = out_pool.tile([P, rows], F32, name="oq",
                                               tag="oq", bufs=4)
                            nc.vector.scalar_tensor_tensor(
                                out=oq[:], in0=xT[3][:], scalar=Mt[q][3][:, h:h + 1],
                                in1=c2[:], op0=AOP.mult, op1=AOP.add)
                            nc.sync.dma_start(
                                out_dram.ap()[q, h * P:(h + 1) * P, s0:s0 + rows],
                                oq[:])

    nc.compile()
    return nc


_BUILD_CACHE = {}


def _get_bass(S, n_cores):
    key = (S, n_cores)
    if key not in _BUILD_CACHE:
        _BUILD_CACHE[key] = build_bass(S, n_cores)
    return _BUILD_CACHE[key]


def _run(x, gamma, beta, trace=False):
    x = np.asarray(x)
    gamma = np.asarray(gamma)
    beta = np.asarray(beta)
    n_cores = 8
    four, B, H, W, Cc = x.shape
    bpc = B // n_cores           # batches per core
    S = bpc * H * W

    gam_t = np.ascontiguousarray(gamma.T.astype(np.float32))
    beta_t = np.ascontiguousarray(beta.T.astype(np.float32))
    ident = np.eye(P, dtype=np.float32)

    in_maps = []
    for k in range(n_cores):
        shard = np.ascontiguousarray(
            x[:, k * bpc:(k + 1) * bpc].reshape(four, S, Cc))
        in_maps.append({"x": shard, "gammaT": gam_t, "betaT": beta_t,
                        "ident": ident})

    nc = _get_bass(S, n_cores)
    res = run_bass_kernel_spmd(nc, in_maps, list(range(n_cores)), trace=trace)

    out = np.empty((four, B, H, W, Cc), dtype=np.float32)
    for k in range(n_cores):
        o = res.results[k]["out_t"]          # [4, C, S]
        out[:, k * bpc:(k + 1) * bpc] = (
            o.transpose(0, 2, 1).reshape(four, bpc, H, W, Cc))
    return out, res


def kernel(x, gamma, beta):
    """x [4,32,56,56,256] f32; gamma [10,256]; beta [4,256] -> [4,32,56,56,256]."""
    out, _ = _run(x, gamma, beta)
    return out


# revision 16
# speedup vs baseline: 173.1417x; 1.0867x over previous
"""Quaternion batch-norm (nn_BatchNormalizationQ) Trainium2 kernel.

Strategy (8 NeuronCores, batch-parallel):
  - Host shards x [4,32,56,56,256] on batch -> per core [4, 12544, 256]
    (component, spatial, channel); gamma/beta pre-transposed to [256,*].
  - Phase 1 (stats): cast-DMA tiles to bf16, PE computes per-channel
    Gram sums  sum_s x_p x_q  (10 pairs) and sums  sum_s x_p  (via
    ones-vector matmuls), accumulated in PSUM.  Diagonals extracted with
    identity-mask multiply + row-reduce.  Partial sums [128,28] are
    AllReduced across the 8 cores.
  - Whitening: per-channel 4x4 inverse-Cholesky + gamma fusion computed
    on-chip on [128,2] tiles (channel on partitions), giving M[4][4] and
    b' = beta - M mu as per-partition scalar vectors.
  - Phase 2 (apply): fp32 tiles, PE-transposed to channel-major strips in
    PSUM, drained to SBUF by ACT; out_q = ((x0*M_q0+b'_q) chained with
    scalar_tensor_tensor FMAs on DVE.  Output written channel-major
    [4, 256, 12544] per core; host transposes back.
"""
import numpy as np

from concourse import bass, bacc, tile, mybir
from concourse.bass_utils import run_bass_kernel_spmd

F32 = mybir.dt.float32
BF16 = mybir.dt.bfloat16
AOP = mybir.AluOpType
AF = mybir.ActivationFunctionType

P = 128
C = 256          # channels
NCOMP = 4        # quaternion components
EPS = 1e-4
CHUNK = 1024     # spatial rows per chunk (must be multiple of 128)

NAMES = "rijk"
TRI = [(p1, p2) for p1 in range(4) for p2 in range(p1, 4)]
TRI_IDX = {}
for _i, (_p, _q) in enumerate(TRI):
    TRI_IDX[(_p, _q)] = _i
    TRI_IDX[(_q, _p)] = _i


def _chunks(S):
    out = []
    s0 = 0
    while s0 < S:
        rows = min(CHUNK, S - s0)
        out.append((s0, rows))
        s0 += rows
    return out


def build_bass(S, n_cores, debug_out=False):
    """Build the SPMD program for per-core spatial size S over n_cores."""
    NTOT = float(S * n_cores)
    nc = bacc.Bacc("TRN2", target_bir_lowering=False, debug=False,
                   num_devices=n_cores)

    x_dram = nc.dram_tensor("x", [NCOMP, S, C], F32, kind="ExternalInput")
    xbf_dram = nc.dram_tensor("xbf", [NCOMP, S, C], BF16, kind="ExternalInput")
    gam_dram = nc.dram_tensor("gammaT", [C, 10], F32, kind="ExternalInput")
    beta_dram = nc.dram_tensor("betaT", [C, NCOMP], F32, kind="ExternalInput")
    id_dram = nc.dram_tensor("ident", [P, P], F32, kind="ExternalInput")
    out_dram = nc.dram_tensor("out_t", [NCOMP, C, S], F32, kind="ExternalOutput")
    if debug_out:
        dbg_stats = nc.dram_tensor("dbg_stats", [P, 28], F32, kind="ExternalOutput")
        dbg_m = nc.dram_tensor("dbg_m", [P, 2, 16], F32, kind="ExternalOutput")
        dbg_bp = nc.dram_tensor("dbg_bp", [P, 2, 4], F32, kind="ExternalOutput")

    chunks = _chunks(S)
    last_ci = len(chunks) - 1

    with tile.TileContext(nc) as tc:
        import contextlib
        stack = contextlib.ExitStack()
        with stack:
            const_pool = stack.enter_context(tc.tile_pool(name="consts", bufs=1))
            wh_pool = stack.enter_context(tc.tile_pool(name="whiten", bufs=1))
            dram_pool = stack.enter_context(
                tc.tile_pool(name="dram", bufs=1, space=bass.MemorySpace.DRAM))

            I = const_pool.tile([P, P], F32, name="I")
            nc.sync.dma_start(I[:], id_dram.ap())
            ones_bf = const_pool.tile([P, 1], BF16, name="ones_bf")
            nc.vector.memset(ones_bf[:], 1.0)
            gam_sb = const_pool.tile([P, 2, 10], F32, name="gam_sb")
            beta_sb = const_pool.tile([P, 2, NCOMP], F32, name="beta_sb")
            for h in range(2):
                nc.sync.dma_start(gam_sb[:, h, :], gam_dram.ap()[h * P:(h + 1) * P, :])
                nc.sync.dma_start(beta_sb[:, h, :], beta_dram.ap()[h * P:(h + 1) * P, :])

            # ---------------- Phase 1: stats ----------------
            # pools stay open (outer stack) so phase-2 allocations get
            # disjoint addresses -> scheduler can prefetch phase-2 loads
            # during phase 1 / the allreduce+whitening bubble
            with (
                tc.tile_pool(name="ph1_psum", bufs=1, space=bass.MemorySpace.PSUM) as pp,
                tc.tile_pool(name="ph1_sbuf", bufs=1) as p1s,
                tc.tile_pool(name="xbf_pool", bufs=1) as xbf_pool,
            ):
                # 20 gram accumulators [128,128] packed 4-per-bank; means [128,8]
                gbank = [pp.tile([P, 512], F32, name=f"gbank{i}") for i in range(5)]
                mbank = pp.tile([P, 8], F32, name="mbank")

                def gslot(t, h):
                    idx = t * 2 + h
                    b, c0 = idx // 4, (idx % 4) * P
                    return gbank[b][:, c0:c0 + P]

                # PSUM start=True zeroes the whole 2KB bank (pending-zero
                # granularity), so emit exactly one start (and one stop) per
                # bank: on the first/last matmul touching it in the fixed
                # (h, p, q) emission order.
                seq = []           # (kind, bank_key)
                for h in range(2):
                    for p in range(NCOMP):
                        seq.append("mbank")
                        for q in range(p, NCOMP):
                            seq.append((TRI_IDX[(p, q)] * 2 + h) // 4)
                first_touch = {}
                last_touch = {}
                for i, b in enumerate(seq):
                    if b not in first_touch:
                        first_touch[b] = i
                    last_touch[b] = i

                for ci, (s0, rows) in enumerate(chunks):
                    nblk = rows // P
                    xbf = []
                    for p in range(NCOMP):
                        t_ = xbf_pool.tile([P, nblk, C], BF16,
                                           name=f"xbf{p}", tag=f"xbf{p}", bufs=2)
                        # partition owns `nblk` consecutive rows (contiguous
                        # nblk*512B per partition); bf16 copy is host-prepared
                        src = xbf_dram.ap()[p, s0:s0 + rows, :].rearrange(
                            "(p m) c -> p m c", p=P)
                        nc.sync.dma_start(t_[:], src)
                        xbf.append(t_)
                    first = ci == 0
                    last = ci == last_ci
                    for m in range(nblk):
                        st_first = first and m == 0
                        st_last = last and m == nblk - 1
                        si = 0
                        for h in range(2):
                            for p in range(NCOMP):
                                st = xbf[p][:, m, h * P:(h + 1) * P]
                                nc.tensor.matmul(
                                    mbank[:, p * 2 + h:p * 2 + h + 1], st, ones_bf[:],
                                    start=st_first and first_touch[seq[si]] == si,
                                    stop=st_last and last_touch[seq[si]] == si,
                                    skip_group_check=True)
                                si += 1
                                for q in range(p, NCOMP):
                                    nc.tensor.matmul(
                                        gslot(TRI_IDX[(p, q)], h), st,
                                        xbf[q][:, m, h * P:(h + 1) * P],
                                        start=st_first and first_touch[seq[si]] == si,
                                        stop=st_last and last_touch[seq[si]] == si,
                                        skip_group_check=True)
                                    si += 1

                # drain stats -> [128, 14, 2] (items: 4 means, 10 gram diags)
                stats_sb = p1s.tile([P, 14, 2], F32, name="stats_sb")
                for p in range(NCOMP):
                    for h in range(2):
                        nc.scalar.copy(stats_sb[:, p, h:h + 1],
                                       mbank[:, p * 2 + h:p * 2 + h + 1])
                for t in range(10):
                    for h in range(2):
                        masked = p1s.tile([P, P], F32, name="masked",
                                          tag="masked", bufs=2)
                        nc.vector.tensor_mul(masked[:], gslot(t, h), I[:])
                        nc.vector.tensor_reduce(
                            out=stats_sb[:, 4 + t, h:h + 1], in_=masked[:],
                            axis=mybir.AxisListType.X, op=AOP.add)

                # AllReduce partial sums across cores
                part_dram = dram_pool.tile([P, 28], F32, name="part_dram")
                cc_dram = dram_pool.tile([P, 28], F32, name="cc_dram",
                                         addr_space="Shared" if n_cores > 4 else "Local")
                nc.scalar.dma_start(part_dram[:], stats_sb[:].rearrange("p a b -> p (a b)"))
                if n_cores > 1:
                    nc.gpsimd.collective_compute(
                        "AllReduce", AOP.add,
                        replica_groups=[list(range(n_cores))],
                        ins=[part_dram.opt()], outs=[cc_dram.opt()])
                    src_stats = cc_dram
                else:
                    src_stats = part_dram
                stats_g = wh_pool.tile([P, 14, 2], F32, name="stats_g")
                nc.scalar.dma_start(stats_g[:].rearrange("p a b -> p (a b)"), src_stats[:])

            # ---------------- whitening math on [128,2] tiles ----------------
            def wt(name):
                return wh_pool.tile([P, 2], F32, name=name, tag=name)

            def vmul(o, a, b):
                nc.vector.tensor_mul(o[:], a[:], b[:])

            def vadd(o, a, b):
                nc.vector.tensor_add(o[:], a[:], b[:])

            def vsub(o, a, b):
                nc.vector.tensor_tensor(o[:], a[:], b[:], AOP.subtract)

            def recip(name, a):
                o = wt(name)
                nc.vector.reciprocal(o[:], a[:])
                return o

            def sqrt_nr(name, v):
                s0 = wt(name + "_s0")
                nc.scalar.sqrt(s0[:], v[:])
                r = recip(name + "_r", s0)
                q = wt(name + "_q")
                vmul(q, v, r)
                s = wt(name + "_s")
                vadd(s, s0, q)
                o = wt(name)
                nc.vector.tensor_scalar_mul(o[:], s[:], 0.5)
                return o

            mu = []
            for p in range(NCOMP):
                m_ = wt(f"mu{p}")
                nc.vector.tensor_scalar_mul(m_[:], stats_g[:, p, :], 1.0 / NTOT)
                mu.append(m_)

            v = {}
            for t, (p, q) in enumerate(TRI):
                name = NAMES[p] + NAMES[q]
                mm = wt(f"mm_{name}")
                vmul(mm, mu[p], mu[q])
                if p == q:
                    nc.vector.tensor_scalar_add(mm[:], mm[:], -EPS)
                vv = wt(f"v_{name}")
                # vv = G/NTOT - (mu_p mu_q - eps_diag)
                nc.vector.scalar_tensor_tensor(
                    out=vv[:], in0=stats_g[:, 4 + t, :], scalar=1.0 / NTOT,
                    in1=mm[:], op0=AOP.mult, op1=AOP.subtract)
                v[name] = vv

            w = {}
            w['rr'] = sqrt_nr("w_rr", v['rr'])
            rc_rr = recip("rc_rr", w['rr'])
            for nm in ('ri', 'rj', 'rk'):
                w[nm] = wt(f"w_{nm}")
                vmul(w[nm], v[nm], rc_rr)
            t1 = wt("t_ii")
            vmul(t1, w['ri'], w['ri'])
            t2 = wt("t_ii2")
            vsub(t2, v['ii'], t1)
            w['ii'] = sqrt_nr("w_ii", t2)
            rc_ii = recip("rc_ii", w['ii'])
            for nm, a, b in (("ij", 'ri', 'rj'), ("ik", 'ri', 'rk')):
                u1 = wt(f"u_{nm}")
                vmul(u1, w[a], w[b])
                u2 = wt(f"u2_{nm}")
                vsub(u2, v[nm], u1)
                w[nm] = wt(f"w_{nm}")
                vmul(w[nm], u2, rc_ii)
            u3 = wt("u_jj")
            vmul(u3, w['ij'], w['ij'])
            u4 = wt("u_jj2")
            vmul(u4, w['rj'], w['rj'])
            u5 = wt("u_jj3")
            vadd(u5, u3, u4)
            u6 = wt("u_jj4")
            vsub(u6, v['jj'], u5)
            w['jj'] = sqrt_nr("w_jj", u6)
            rc_jj = recip("rc_jj", w['jj'])
            u7 = wt("u_jk")
            vmul(u7, w['ij'], w['ik'])
            u8 = wt("u_jk2")
            vmul(u8, w['rj'], w['rk'])
            u9 = wt("u_jk3")
            vadd(u9, u7, u8)
            u10 = wt("u_jk4")
            vsub(u10, v['jk'], u9)
            w['jk'] = wt("w_jk")
            vmul(w['jk'], u10, rc_jj)
            u11 = wt("u_kk")
            vmul(u11, w['jk'], w['jk'])
            u12 = wt("u_kk2")
            vmul(u12, w['ik'], w['ik'])
            u13 = wt("u_kk3")
            vadd(u13, u11, u12)
            u14 = wt("u_kk4")
            vmul(u14, w['rk'], w['rk'])
            u15 = wt("u_kk5")
            vadd(u15, u13, u14)
            u16 = wt("u_kk6")
            vsub(u16, v['kk'], u15)
            w['kk'] = sqrt_nr("w_kk", u16)
            rc_kk = recip("rc_kk", w['kk'])

            o = {}
            o['rr'], o['ii'], o['jj'], o['kk'] = rc_rr, rc_ii, rc_jj, rc_kk

            def neg_mul(name, a, b, rc):
                # returns -(a*b)*rc
                z1 = wt(name + "_z1")
                vmul(z1, a, b)
                z2 = wt(name + "_z2")
                vmul(z2, z1, rc)
                z3 = wt(name)
                nc.vector.tensor_scalar_mul(z3[:], z2[:], -1.0)
                return z3

            o['ri'] = neg_mul("o_ri", w['ri'], o['rr'], rc_ii)
            z1 = wt("ork_a")
            vmul(z1, w['rj'], o['rr'])
            z2 = wt("ork_b")
            vmul(z2, w['ij'], o['ri'])
            z3 = wt("ork_c")
            vadd(z3, z1, z2)
            z4 = wt("ork_d")
            vmul(z4, z3, rc_jj)
            o['rj'] = wt("o_rj")
            nc.vector.tensor_scalar_mul(o['rj'][:], z4[:], -1.0)
            y1 = wt("orkk_a")
            vmul(y1, w['rk'], o['rr'])
            y2 = wt("orkk_b")
            vmul(y2, w['ik'], o['ri'])
            y3 = wt("orkk_c")
            vmul(y3, w['jk'], o['rj'])
            y4 = wt("orkk_d")
            vadd(y4, y1, y2)
            y5 = wt("orkk_e")
            vadd(y5, y4, y3)
            y6 = wt("orkk_f")
            vmul(y6, y5, rc_kk)
            o['rk'] = wt("o_rk")
            nc.vector.tensor_scalar_mul(o['rk'][:], y6[:], -1.0)
            o['ij'] = neg_mul("o_ij", w['ij'], o['ii'], rc_jj)
            x1 = wt("oik_a")
            vmul(x1, w['ik'], o['ii'])
            x2 = wt("oik_b")
            vmul(x2, w['jk'], o['ij'])
            x3 = wt("oik_c")
            vadd(x3, x1, x2)
            x4 = wt("oik_d")
            vmul(x4, x3, rc_kk)
            o['ik'] = wt("o_ik")
            nc.vector.tensor_scalar_mul(o['ik'][:], x4[:], -1.0)
            o['jk'] = neg_mul("o_jk", w['jk'], o['jj'], rc_kk)

            def Wsym(a, b):
                i1, i2 = min(a, b), max(a, b)
                return o[NAMES[i1] + NAMES[i2]]

            def Gsym(a, b):
                return gam_sb[:, :, TRI_IDX[(a, b)]]

            # M[p][q] = sum_s G(p,s) W(s,q); bprime[p] = beta_p - sum_q M[p][q] mu_q
            Mt = [[None] * NCOMP for _ in range(NCOMP)]
            bp = [None] * NCOMP
            for p in range(NCOMP):
                for q in range(NCOMP):
                    acc = wh_pool.tile([P, 2], F32, name=f"M{p}{q}", tag=f"M{p}{q}")
                    tmp0 = wt(f"Mt{p}{q}_0")
                    vmul(tmp0, Gsym(p, 0), Wsym(0, q))
                    nc.vector.tensor_copy(acc[:], tmp0[:])
                    for s_ in range(1, NCOMP):
                        tmp = wt(f"Mt{p}{q}_{s_}")
                        vmul(tmp, Gsym(p, s_), Wsym(s_, q))
                        vadd(acc, acc, tmp)
                    Mt[p][q] = acc
                bacc_t = wh_pool.tile([P, 2], F32, name=f"bp{p}", tag=f"bp{p}")
                nc.vector.tensor_copy(bacc_t[:], beta_sb[:, :, p])
                for q in range(NCOMP):
                    tmp = wt(f"bp{p}_{q}")
                    vmul(tmp, Mt[p][q], mu[q])
                    vsub(bacc_t, bacc_t, tmp)
                bp[p] = bacc_t

            if debug_out:
                nc.sync.dma_start(dbg_stats.ap(), stats_g[:].rearrange("p a b -> p (a b)"))
                for p_ in range(NCOMP):
                    for q_ in range(NCOMP):
                        nc.sync.dma_start(dbg_m.ap()[:, :, p_ * 4 + q_], Mt[p_][q_][:])
                    nc.sync.dma_start(dbg_bp.ap()[:, :, p_], bp[p_][:])

            # ---------------- Phase 2: apply ----------------
            # out_q = sum_s M[q][s] x_s + b'_q, computed as four 2x-mode DVE
            # tensor_scalar products (rounded to float32r) merged by PE
            # identity-matmuls accumulating in PSUM (float32r streams at
            # 1 cyc/row), drained by ACT.
            with (
                tc.tile_pool(name="xf_pool", bufs=1) as xf_pool,
                tc.tile_pool(name="strip_psum", bufs=1, space=bass.MemorySpace.PSUM) as sp,
                tc.tile_pool(name="macc_psum", bufs=1, space=bass.MemorySpace.PSUM) as mp,
                tc.tile_pool(name="xT_pool", bufs=1) as xT_pool,
                tc.tile_pool(name="chain_pool", bufs=1) as chain_pool,
                tc.tile_pool(name="out_pool", bufs=1) as out_pool,
            ):
                Ir = const_pool.tile([P, P], mybir.dt.float32r, name="Ir")
                nc.vector.tensor_copy(Ir[:], I[:])
                for ci, (s0, rows) in enumerate(chunks):
                    nblk = rows // P
                    xf = []
                    for p in range(NCOMP):
                        t_ = xf_pool.tile([P, nblk, C], F32,
                                          name=f"xf{p}", tag=f"xf{p}", bufs=2)
                        # partition = row within each 128-block (transposable)
                        src = x_dram.ap()[p, s0:s0 + rows, :].rearrange(
                            "(m p) c -> p m c", p=P)
                        nc.sync.dma_start(t_[:], src)
                        xf.append(t_)
                    for h in range(2):
                        xT = []
                        for p in range(NCOMP):
                            xt = xT_pool.tile([P, rows], F32,
                                              name=f"xT{p}", tag=f"xT{p}", bufs=2)
                            m0 = 0
                            while m0 < nblk:
                                g = min(4, nblk - m0)
                                wdt = g * P
                                strip = sp.tile([P, 512], F32, name="strip",
                                                tag="strip", bufs=3)
                                for ji in range(g):
                                    nc.tensor.transpose(
                                        strip[:, ji * P:(ji + 1) * P],
                                        xf[p][:, m0 + ji, h * P:(h + 1) * P], I[:])
                                nc.scalar.copy(xt[:, m0 * P:m0 * P + wdt],
                                               strip[:, 0:wdt])
                                m0 += g
                            xT.append(xt)
                        for q in range(NCOMP):
                            ds = []
                            for s_ in range(NCOMP):
                                d_ = chain_pool.tile([P, rows], mybir.dt.float32r,
                                                     name=f"d{s_}", tag=f"d{s_}",
                                                     bufs=3)
                                if s_ == 0:
                                    nc.vector.tensor_scalar(
                                        d_[:], xT[0][:], Mt[q][0][:, h:h + 1],
                                        bp[q][:, h:h + 1], AOP.mult, AOP.add)
                                else:
                                    nc.vector.tensor_scalar_mul(
                                        d_[:], xT[s_][:], Mt[q][s_][:, h:h + 1])
                                ds.append(d_)
                            oq = out_pool.tile([P, rows], F32, name="oq",
                                               tag="oq", bufs=4)
                            m0 = 0
                            while m0 < rows:
                                ws = min(512, rows - m0)
                                macc = mp.tile([P, 512], F32, name="macc",
                                               tag="macc", bufs=4)
                                for s_ in range(NCOMP):
                                    nc.tensor.matmul(
                                        macc[:, 0:ws], Ir[:], ds[s_][:, m0:m0 + ws],
                                        start=(s_ == 0), stop=(s_ == NCOMP - 1),
                                        skip_group_check=True)
                                nc.scalar.copy(oq[:, m0:m0 + ws], macc[:, 0:ws])
                                m0 += ws
                            nc.sync.dma_start(
                                out_dram.ap()[q, h * P:(h + 1) * P, s0:s0 + rows],
                                oq[:])

    nc.compile()
    return nc


_BUILD_CACHE = {}


def _get_bass(S, n_cores):
    key = (S, n_cores)
    if key not in _BUILD_CACHE:
        _BUILD_CACHE[key] = build_bass(S, n_cores)
    return _BUILD_CACHE[key]


def _run(x, gamma, beta, trace=False):
    x = np.asarray(x)
    gamma = np.asarray(gamma)
    beta = np.asarray(beta)
    n_cores = 8
    four, B, H, W, Cc = x.shape
    bpc = B // n_cores           # batches per core
    S = bpc * H * W

    gam_t = np.ascontiguousarray(gamma.T.astype(np.float32))
    beta_t = np.ascontiguousarray(beta.T.astype(np.float32))
    ident = np.eye(P, dtype=np.float32)

    import ml_dtypes
    in_maps = []
    for k in range(n_cores):
        shard = np.ascontiguousarray(
            x[:, k * bpc:(k + 1) * bpc].reshape(four, S, Cc))
        in_maps.append({"x": shard, "xbf": shard.astype(ml_dtypes.bfloat16),
                        "gammaT": gam_t, "betaT": beta_t, "ident": ident})

    nc = _get_bass(S, n_cores)
    res = run_bass_kernel_spmd(nc, in_maps, list(range(n_cores)), trace=trace)

    out = np.empty((four, B, H, W, Cc), dtype=np.float32)
    for k in range(n_cores):
        o = res.results[k]["out_t"]          # [4, C, S]
        out[:, k * bpc:(k + 1) * bpc] = (
            o.transpose(0, 2, 1).reshape(four, bpc, H, W, Cc))
    return out, res


def kernel(x, gamma, beta):
    """x [4,32,56,56,256] f32; gamma [10,256]; beta [4,256] -> [4,32,56,56,256]."""
    out, _ = _run(x, gamma, beta)
    return out


# revision 20
# speedup vs baseline: 177.6875x; 1.0263x over previous
"""Quaternion batch-norm (nn_BatchNormalizationQ) Trainium2 kernel.

Strategy (8 NeuronCores, batch-parallel):
  - Host shards x [4,32,56,56,256] on batch -> per core [4, 12544, 256]
    (component, spatial, channel); gamma/beta pre-transposed to [256,*].
  - Phase 1 (stats): cast-DMA tiles to bf16, PE computes per-channel
    Gram sums  sum_s x_p x_q  (10 pairs) and sums  sum_s x_p  (via
    ones-vector matmuls), accumulated in PSUM.  Diagonals extracted with
    identity-mask multiply + row-reduce.  Partial sums [128,28] are
    AllReduced across the 8 cores.
  - Whitening: per-channel 4x4 inverse-Cholesky + gamma fusion computed
    on-chip on [128,2] tiles (channel on partitions), giving M[4][4] and
    b' = beta - M mu as per-partition scalar vectors.
  - Phase 2 (apply): fp32 tiles, PE-transposed to channel-major strips in
    PSUM, drained to SBUF by ACT; out_q = ((x0*M_q0+b'_q) chained with
    scalar_tensor_tensor FMAs on DVE.  Output written channel-major
    [4, 256, 12544] per core; host transposes back.
"""
import numpy as np

from concourse import bass, bacc, tile, mybir
from concourse.bass_utils import run_bass_kernel_spmd

F32 = mybir.dt.float32
BF16 = mybir.dt.bfloat16
AOP = mybir.AluOpType
AF = mybir.ActivationFunctionType

P = 128
C = 256          # channels
NCOMP = 4        # quaternion components
EPS = 1e-4
CHUNK = 1024     # spatial rows per chunk (must be multiple of 128)

NAMES = "rijk"
TRI = [(p1, p2) for p1 in range(4) for p2 in range(p1, 4)]
TRI_IDX = {}
for _i, (_p, _q) in enumerate(TRI):
    TRI_IDX[(_p, _q)] = _i
    TRI_IDX[(_q, _p)] = _i


def _chunks(S):
    out = []
    s0 = 0
    while s0 < S:
        rows = min(CHUNK, S - s0)
        out.append((s0, rows))
        s0 += rows
    return out


def build_bass(S, n_cores, debug_out=False):
    """Build the SPMD program for per-core spatial size S over n_cores."""
    NTOT = float(S * n_cores)
    nc = bacc.Bacc("TRN2", target_bir_lowering=False, debug=False,
                   num_devices=n_cores)

    x_dram = nc.dram_tensor("x", [NCOMP, S, C], F32, kind="ExternalInput")
    xbf_dram = nc.dram_tensor("xbf", [NCOMP, S, C], BF16, kind="ExternalInput")
    gam_dram = nc.dram_tensor("gammaT", [C, 10], F32, kind="ExternalInput")
    beta_dram = nc.dram_tensor("betaT", [C, NCOMP], F32, kind="ExternalInput")
    id_dram = nc.dram_tensor("ident", [P, P], F32, kind="ExternalInput")
    out_dram = nc.dram_tensor("out_t", [NCOMP, C, S], F32, kind="ExternalOutput")
    if debug_out:
        dbg_stats = nc.dram_tensor("dbg_stats", [P, 28], F32, kind="ExternalOutput")
        dbg_m = nc.dram_tensor("dbg_m", [P, 2, 16], F32, kind="ExternalOutput")
        dbg_bp = nc.dram_tensor("dbg_bp", [P, 2, 4], F32, kind="ExternalOutput")

    chunks = _chunks(S)
    last_ci = len(chunks) - 1

    with tile.TileContext(nc) as tc:
        import contextlib
        stack = contextlib.ExitStack()
        with stack:
            const_pool = stack.enter_context(tc.tile_pool(name="consts", bufs=1))
            wh_pool = stack.enter_context(tc.tile_pool(name="whiten", bufs=1))
            dram_pool = stack.enter_context(
                tc.tile_pool(name="dram", bufs=1, space=bass.MemorySpace.DRAM))

            I = const_pool.tile([P, P], F32, name="I")
            nc.sync.dma_start(I[:], id_dram.ap())
            ones_bf = const_pool.tile([P, 1], BF16, name="ones_bf")
            nc.vector.memset(ones_bf[:], 1.0)
            gam_sb = const_pool.tile([P, 2, 10], F32, name="gam_sb")
            beta_sb = const_pool.tile([P, 2, NCOMP], F32, name="beta_sb")
            for h in range(2):
                nc.sync.dma_start(gam_sb[:, h, :], gam_dram.ap()[h * P:(h + 1) * P, :])
                nc.sync.dma_start(beta_sb[:, h, :], beta_dram.ap()[h * P:(h + 1) * P, :])

            # phase-2 input pool allocated up front: its addresses are
            # disjoint from phase-1 tiles, so the first apply-phase loads can
            # prefetch during phase 1 and the allreduce/whitening bubble
            xf_pool = stack.enter_context(tc.tile_pool(name="xf_pool", bufs=1))

            # ---------------- Phase 1: stats ----------------
            with (
                tc.tile_pool(name="ph1_psum", bufs=1, space=bass.MemorySpace.PSUM) as pp,
                tc.tile_pool(name="ph1_sbuf", bufs=1) as p1s,
                tc.tile_pool(name="xbf_pool", bufs=1) as xbf_pool,
            ):
                # 20 gram accumulators [128,128] packed 4-per-bank; means [128,8]
                gbank = [pp.tile([P, 512], F32, name=f"gbank{i}") for i in range(5)]
                mbank = pp.tile([P, 8], F32, name="mbank")

                def gslot(t, h):
                    idx = t * 2 + h
                    b, c0 = idx // 4, (idx % 4) * P
                    return gbank[b][:, c0:c0 + P]

                # PSUM start=True zeroes the whole 2KB bank (pending-zero
                # granularity), so emit exactly one start (and one stop) per
                # bank: on the first/last matmul touching it in the fixed
                # (h, p, q) emission order.
                seq = []           # (kind, bank_key)
                for h in range(2):
                    for p in range(NCOMP):
                        seq.append("mbank")
                        for q in range(p, NCOMP):
                            seq.append((TRI_IDX[(p, q)] * 2 + h) // 4)
                first_touch = {}
                last_touch = {}
                for i, b in enumerate(seq):
                    if b not in first_touch:
                        first_touch[b] = i
                    last_touch[b] = i

                for ci, (s0, rows) in enumerate(chunks):
                    nblk = rows // P
                    xbf = []
                    for p in range(NCOMP):
                        t_ = xbf_pool.tile([P, nblk, C], BF16,
                                           name=f"xbf{p}", tag=f"xbf{p}", bufs=2)
                        # partition owns `nblk` consecutive rows (contiguous
                        # nblk*512B per partition); bf16 copy is host-prepared
                        src = xbf_dram.ap()[p, s0:s0 + rows, :].rearrange(
                            "(p m) c -> p m c", p=P)
                        nc.sync.dma_start(t_[:], src)
                        xbf.append(t_)
                    first = ci == 0
                    last = ci == last_ci
                    for m in range(nblk):
                        st_first = first and m == 0
                        st_last = last and m == nblk - 1
                        si = 0
                        for h in range(2):
                            for p in range(NCOMP):
                                st = xbf[p][:, m, h * P:(h + 1) * P]
                                nc.tensor.matmul(
                                    mbank[:, p * 2 + h:p * 2 + h + 1], st, ones_bf[:],
                                    start=st_first and first_touch[seq[si]] == si,
                                    stop=st_last and last_touch[seq[si]] == si,
                                    skip_group_check=True)
                                si += 1
                                for q in range(p, NCOMP):
                                    nc.tensor.matmul(
                                        gslot(TRI_IDX[(p, q)], h), st,
                                        xbf[q][:, m, h * P:(h + 1) * P],
                                        start=st_first and first_touch[seq[si]] == si,
                                        stop=st_last and last_touch[seq[si]] == si,
                                        skip_group_check=True)
                                    si += 1

                # drain stats -> [128, 14, 2] (items: 4 means, 10 gram diags)
                stats_sb = p1s.tile([P, 14, 2], F32, name="stats_sb")
                for p in range(NCOMP):
                    for h in range(2):
                        nc.scalar.copy(stats_sb[:, p, h:h + 1],
                                       mbank[:, p * 2 + h:p * 2 + h + 1])
                for t in range(10):
                    for h in range(2):
                        masked = p1s.tile([P, P], F32, name="masked",
                                          tag="masked", bufs=2)
                        nc.vector.tensor_mul(masked[:], gslot(t, h), I[:])
                        nc.vector.tensor_reduce(
                            out=stats_sb[:, 4 + t, h:h + 1], in_=masked[:],
                            axis=mybir.AxisListType.X, op=AOP.add)

                # AllReduce partial sums across cores
                part_dram = dram_pool.tile([P, 28], F32, name="part_dram")
                cc_dram = dram_pool.tile([P, 28], F32, name="cc_dram",
                                         addr_space="Shared" if n_cores > 4 else "Local")
                nc.scalar.dma_start(part_dram[:], stats_sb[:].rearrange("p a b -> p (a b)"))
                if n_cores > 1:
                    nc.gpsimd.collective_compute(
                        "AllReduce", AOP.add,
                        replica_groups=[list(range(n_cores))],
                        ins=[part_dram.opt()], outs=[cc_dram.opt()])
                    src_stats = cc_dram
                else:
                    src_stats = part_dram
                stats_g = wh_pool.tile([P, 14, 2], F32, name="stats_g")
                nc.scalar.dma_start(stats_g[:].rearrange("p a b -> p (a b)"), src_stats[:])

            # ---------------- whitening math on [128,2] tiles ----------------
            def wt(name):
                return wh_pool.tile([P, 2], F32, name=name, tag=name)

            def vmul(o, a, b):
                nc.vector.tensor_mul(o[:], a[:], b[:])

            def vadd(o, a, b):
                nc.vector.tensor_add(o[:], a[:], b[:])

            def vsub(o, a, b):
                nc.vector.tensor_tensor(o[:], a[:], b[:], AOP.subtract)

            def recip(name, a):
                o = wt(name)
                nc.vector.reciprocal(o[:], a[:])
                return o

            def sqrt_nr(name, v):
                s0 = wt(name + "_s0")
                nc.scalar.sqrt(s0[:], v[:])
                r = recip(name + "_r", s0)
                q = wt(name + "_q")
                vmul(q, v, r)
                s = wt(name + "_s")
                vadd(s, s0, q)
                o = wt(name)
                nc.vector.tensor_scalar_mul(o[:], s[:], 0.5)
                return o

            mu = []
            for p in range(NCOMP):
                m_ = wt(f"mu{p}")
                nc.vector.tensor_scalar_mul(m_[:], stats_g[:, p, :], 1.0 / NTOT)
                mu.append(m_)

            v = {}
            for t, (p, q) in enumerate(TRI):
                name = NAMES[p] + NAMES[q]
                mm = wt(f"mm_{name}")
                vmul(mm, mu[p], mu[q])
                if p == q:
                    nc.vector.tensor_scalar_add(mm[:], mm[:], -EPS)
                vv = wt(f"v_{name}")
                # vv = G/NTOT - (mu_p mu_q - eps_diag)
                nc.vector.scalar_tensor_tensor(
                    out=vv[:], in0=stats_g[:, 4 + t, :], scalar=1.0 / NTOT,
                    in1=mm[:], op0=AOP.mult, op1=AOP.subtract)
                v[name] = vv

            w = {}
            w['rr'] = sqrt_nr("w_rr", v['rr'])
            rc_rr = recip("rc_rr", w['rr'])
            for nm in ('ri', 'rj', 'rk'):
                w[nm] = wt(f"w_{nm}")
                vmul(w[nm], v[nm], rc_rr)
            t1 = wt("t_ii")
            vmul(t1, w['ri'], w['ri'])
            t2 = wt("t_ii2")
            vsub(t2, v['ii'], t1)
            w['ii'] = sqrt_nr("w_ii", t2)
            rc_ii = recip("rc_ii", w['ii'])
            for nm, a, b in (("ij", 'ri', 'rj'), ("ik", 'ri', 'rk')):
                u1 = wt(f"u_{nm}")
                vmul(u1, w[a], w[b])
                u2 = wt(f"u2_{nm}")
                vsub(u2, v[nm], u1)
                w[nm] = wt(f"w_{nm}")
                vmul(w[nm], u2, rc_ii)
            u3 = wt("u_jj")
            vmul(u3, w['ij'], w['ij'])
            u4 = wt("u_jj2")
            vmul(u4, w['rj'], w['rj'])
            u5 = wt("u_jj3")
            vadd(u5, u3, u4)
            u6 = wt("u_jj4")
            vsub(u6, v['jj'], u5)
            w['jj'] = sqrt_nr("w_jj", u6)
            rc_jj = recip("rc_jj", w['jj'])
            u7 = wt("u_jk")
            vmul(u7, w['ij'], w['ik'])
            u8 = wt("u_jk2")
            vmul(u8, w['rj'], w['rk'])
            u9 = wt("u_jk3")
            vadd(u9, u7, u8)
            u10 = wt("u_jk4")
            vsub(u10, v['jk'], u9)
            w['jk'] = wt("w_jk")
            vmul(w['jk'], u10, rc_jj)
            u11 = wt("u_kk")
            vmul(u11, w['jk'], w['jk'])
            u12 = wt("u_kk2")
            vmul(u12, w['ik'], w['ik'])
            u13 = wt("u_kk3")
            vadd(u13, u11, u12)
            u14 = wt("u_kk4")
            vmul(u14, w['rk'], w['rk'])
            u15 = wt("u_kk5")
            vadd(u15, u13, u14)
            u16 = wt("u_kk6")
            vsub(u16, v['kk'], u15)
            w['kk'] = sqrt_nr("w_kk", u16)
            rc_kk = recip("rc_kk", w['kk'])

            o = {}
            o['rr'], o['ii'], o['jj'], o['kk'] = rc_rr, rc_ii, rc_jj, rc_kk

            def neg_mul(name, a, b, rc):
                # returns -(a*b)*rc
                z1 = wt(name + "_z1")
                vmul(z1, a, b)
                z2 = wt(name + "_z2")
                vmul(z2, z1, rc)
                z3 = wt(name)
                nc.vector.tensor_scalar_mul(z3[:], z2[:], -1.0)
                return z3

            o['ri'] = neg_mul("o_ri", w['ri'], o['rr'], rc_ii)
            z1 = wt("ork_a")
            vmul(z1, w['rj'], o['rr'])
            z2 = wt("ork_b")
            vmul(z2, w['ij'], o['ri'])
            z3 = wt("ork_c")
            vadd(z3, z1, z2)
            z4 = wt("ork_d")
            vmul(z4, z3, rc_jj)
            o['rj'] = wt("o_rj")
            nc.vector.tensor_scalar_mul(o['rj'][:], z4[:], -1.0)
            y1 = wt("orkk_a")
            vmul(y1, w['rk'], o['rr'])
            y2 = wt("orkk_b")
            vmul(y2, w['ik'], o['ri'])
            y3 = wt("orkk_c")
            vmul(y3, w['jk'], o['rj'])
            y4 = wt("orkk_d")
            vadd(y4, y1, y2)
            y5 = wt("orkk_e")
            vadd(y5, y4, y3)
            y6 = wt("orkk_f")
            vmul(y6, y5, rc_kk)
            o['rk'] = wt("o_rk")
            nc.vector.tensor_scalar_mul(o['rk'][:], y6[:], -1.0)
            o['ij'] = neg_mul("o_ij", w['ij'], o['ii'], rc_jj)
            x1 = wt("oik_a")
            vmul(x1, w['ik'], o['ii'])
            x2 = wt("oik_b")
            vmul(x2, w['jk'], o['ij'])
            x3 = wt("oik_c")
            vadd(x3, x1, x2)
            x4 = wt("oik_d")
            vmul(x4, x3, rc_kk)
            o['ik'] = wt("o_ik")
            nc.vector.tensor_scalar_mul(o['ik'][:], x4[:], -1.0)
            o['jk'] = neg_mul("o_jk", w['jk'], o['jj'], rc_kk)

            def Wsym(a, b):
                i1, i2 = min(a, b), max(a, b)
                return o[NAMES[i1] + NAMES[i2]]

            def Gsym(a, b):
                return gam_sb[:, :, TRI_IDX[(a, b)]]

            # M[p][q] = sum_s G(p,s) W(s,q); bprime[p] = beta_p - sum_q M[p][q] mu_q
            Mt = [[None] * NCOMP for _ in range(NCOMP)]
            bp = [None] * NCOMP
            for p in range(NCOMP):
                for q in range(NCOMP):
                    acc = wh_pool.tile([P, 2], F32, name=f"M{p}{q}", tag=f"M{p}{q}")
                    tmp0 = wt(f"Mt{p}{q}_0")
                    vmul(tmp0, Gsym(p, 0), Wsym(0, q))
                    nc.vector.tensor_copy(acc[:], tmp0[:])
                    for s_ in range(1, NCOMP):
                        tmp = wt(f"Mt{p}{q}_{s_}")
                        vmul(tmp, Gsym(p, s_), Wsym(s_, q))
                        vadd(acc, acc, tmp)
                    Mt[p][q] = acc
                bacc_t = wh_pool.tile([P, 2], F32, name=f"bp{p}", tag=f"bp{p}")
                nc.vector.tensor_copy(bacc_t[:], beta_sb[:, :, p])
                for q in range(NCOMP):
                    tmp = wt(f"bp{p}_{q}")
                    vmul(tmp, Mt[p][q], mu[q])
                    vsub(bacc_t, bacc_t, tmp)
                bp[p] = bacc_t

            if debug_out:
                nc.sync.dma_start(dbg_stats.ap(), stats_g[:].rearrange("p a b -> p (a b)"))
                for p_ in range(NCOMP):
                    for q_ in range(NCOMP):
                        nc.sync.dma_start(dbg_m.ap()[:, :, p_ * 4 + q_], Mt[p_][q_][:])
                    nc.sync.dma_start(dbg_bp.ap()[:, :, p_], bp[p_][:])

            # ---------------- Phase 2: apply ----------------
            # out_q = sum_s M[q][s] x_s + b'_q, computed as four 2x-mode DVE
            # tensor_scalar products (rounded to float32r) merged by PE
            # identity-matmuls accumulating in PSUM (float32r streams at
            # 1 cyc/row), drained by ACT.
            with (
                tc.tile_pool(name="strip_psum", bufs=1, space=bass.MemorySpace.PSUM) as sp,
                tc.tile_pool(name="macc_psum", bufs=1, space=bass.MemorySpace.PSUM) as mp,
                tc.tile_pool(name="xT_pool", bufs=1) as xT_pool,
                tc.tile_pool(name="chain_pool", bufs=1) as chain_pool,
                tc.tile_pool(name="out_pool", bufs=1) as out_pool,
            ):
                Ir = const_pool.tile([P, P], mybir.dt.float32r, name="Ir")
                nc.vector.tensor_copy(Ir[:], I[:])
                for ci, (s0, rows) in enumerate(chunks):
                    nblk = rows // P
                    xf = []
                    for p in range(NCOMP):
                        t_ = xf_pool.tile([P, nblk, C], F32,
                                          name=f"xf{p}", tag=f"xf{p}", bufs=2)
                        # partition = row within each 128-block (transposable)
                        src = x_dram.ap()[p, s0:s0 + rows, :].rearrange(
                            "(m p) c -> p m c", p=P)
                        nc.sync.dma_start(t_[:], src)
                        xf.append(t_)
                    for h in range(2):
                        xT = []
                        for p in range(NCOMP):
                            xt = xT_pool.tile([P, rows], F32,
                                              name=f"xT{p}", tag=f"xT{p}", bufs=2)
                            m0 = 0
                            while m0 < nblk:
                                g = min(4, nblk - m0)
                                wdt = g * P
                                strip = sp.tile([P, 512], F32, name="strip",
                                                tag="strip", bufs=3)
                                for ji in range(g):
                                    nc.tensor.transpose(
                                        strip[:, ji * P:(ji + 1) * P],
                                        xf[p][:, m0 + ji, h * P:(h + 1) * P], I[:])
                                nc.scalar.copy(xt[:, m0 * P:m0 * P + wdt],
                                               strip[:, 0:wdt])
                                m0 += g
                            xT.append(xt)
                        for q in range(NCOMP):
                            ds = []
                            for s_ in range(NCOMP):
                                d_ = chain_pool.tile([P, rows], mybir.dt.float32r,
                                                     name=f"d{s_}", tag=f"d{s_}",
                                                     bufs=3)
                                if s_ == 0:
                                    nc.vector.tensor_scalar(
                                        d_[:], xT[0][:], Mt[q][0][:, h:h + 1],
                                        bp[q][:, h:h + 1], AOP.mult, AOP.add)
                                else:
                                    nc.vector.tensor_scalar_mul(
                                        d_[:], xT[s_][:], Mt[q][s_][:, h:h + 1])
                                ds.append(d_)
                            oq = out_pool.tile([P, rows], F32, name="oq",
                                               tag="oq", bufs=4)
                            m0 = 0
                            while m0 < rows:
                                ws = min(512, rows - m0)
                                macc = mp.tile([P, 512], F32, name="macc",
                                               tag="macc", bufs=4)
                                for s_ in range(NCOMP):
                                    nc.tensor.matmul(
                                        macc[:, 0:ws], Ir[:], ds[s_][:, m0:m0 + ws],
                                        start=(s_ == 0), stop=(s_ == NCOMP - 1),
                                        skip_group_check=True)
                                nc.scalar.copy(oq[:, m0:m0 + ws], macc[:, 0:ws])
                                m0 += ws
                            nc.sync.dma_start(
                                out_dram.ap()[q, h * P:(h + 1) * P, s0:s0 + rows],
                                oq[:])

    nc.compile()
    return nc


_BUILD_CACHE = {}


def _get_bass(S, n_cores):
    key = (S, n_cores)
    if key not in _BUILD_CACHE:
        _BUILD_CACHE[key] = build_bass(S, n_cores)
    return _BUILD_CACHE[key]


def _run(x, gamma, beta, trace=False):
    x = np.asarray(x)
    gamma = np.asarray(gamma)
    beta = np.asarray(beta)
    n_cores = 8
    four, B, H, W, Cc = x.shape
    bpc = B // n_cores           # batches per core
    S = bpc * H * W

    gam_t = np.ascontiguousarray(gamma.T.astype(np.float32))
    beta_t = np.ascontiguousarray(beta.T.astype(np.float32))
    ident = np.eye(P, dtype=np.float32)

    import ml_dtypes
    in_maps = []
    for k in range(n_cores):
        shard = np.ascontiguousarray(
            x[:, k * bpc:(k + 1) * bpc].reshape(four, S, Cc))
        in_maps.append({"x": shard, "xbf": shard.astype(ml_dtypes.bfloat16),
                        "gammaT": gam_t, "betaT": beta_t, "ident": ident})

    nc = _get_bass(S, n_cores)
    res = run_bass_kernel_spmd(nc, in_maps, list(range(n_cores)), trace=trace)

    out = np.empty((four, B, H, W, Cc), dtype=np.float32)
    for k in range(n_cores):
        o = res.results[k]["out_t"]          # [4, C, S]
        out[:, k * bpc:(k + 1) * bpc] = (
            o.transpose(0, 2, 1).reshape(four, bpc, H, W, Cc))
    return out, res


def kernel(x, gamma, beta):
    """x [4,32,56,56,256] f32; gamma [10,256]; beta [4,256] -> [4,32,56,56,256]."""
    out, _ = _run(x, gamma, beta)
    return out
